# revision 6
# baseline (speedup 1.0000x reference)
"""DCGRU cell Trainium2 kernel v3 (8 NeuronCores, node-sharded SpMM).

Core c owns dest nodes [1250c, 1250(c+1)) as 10 tiles of 128 (zero-padded).
Diffusion gathers read fp8(e3m4) token tables (token = [B=32, 72] for the
x-path = 2304 B/row, [B, 64] halves of a 4096 B/row table for the
projected cand path), halving the dominant HBM gather traffic vs fp16;
M matrices stay fp16 (mixed-dtype matmuls, fp32 PSUM accumulate).

Projections run in the transposed (feature-on-partition) domain:
identity-matmul transposes (cheaper than PE transpose-mode and they keep
the HAM clock-gate warm), stationary folded weight blocks with N=512
moving operands, sigmoid with per-partition bias on the scalar engine,
and paired back-transposes for the node-major y tables.

Chebyshev 2x/-x0 terms fold into the gate/cand weight blocks so one
index+M structure per support serves all 4 of its SpMM uses.  z0|z1
share one AllGather.  Explicit dep edges guard every collective.
"""
import sys

sys.path.insert(0, '/opt/trn_rl_repo')

import numpy as np
import ml_dtypes

F8NP = ml_dtypes.float8_e3m4

N = 10000
U = 64
DIN = 2
B = 32
NCORE = 8
SHARD = N // NCORE            # 1250
TILE = 128
TPC = 10                      # tiles per core (1250 -> 10x128 padded)
RT = TPC * TILE               # 1280 padded rows per shard
FULL = RT * NCORE             # 10240 table rows
F = DIN + U                   # 66
FB = 72                       # per-batch stride in x tokens (66 + 6 pad)
WG = B * FB                   # 2304 x-path token elems (fp8: 2304 B, %256)
WC = B * U                    # 2048 cand-path token elems
NM = 5

HALF = NCORE * 5 * TILE       # 5120 rows per table half

_CACHE = {}


def _grow(n):
    c = n // SHARD
    loc = n % SHARD
    t = loc // TILE
    return (t // 5) * HALF + c * (5 * TILE) + (t % 5) * TILE + loc % TILE


def _pack_tok_idx(vec):
    flat = np.asarray(vec, np.int16)
    M = len(flat)
    assert M % 16 == 0
    i = np.arange(M)
    buf = np.zeros((128, M // 16), np.int16)
    for g in range(8):
        buf[(i % 16) + 16 * g, i // 16] = flat
    return buf


def _build_support(rows, cols, vals):
    """Per-core per-tile unique-source token lists + M matrices."""
    core = rows // SHARD
    loc = rows % SHARD
    tile = loc // TILE
    dstl = loc % TILE
    gsrc = _grow(cols)
    key = core * TPC + tile
    order = np.argsort(key, kind='stable')
    ks, gs, ds, vs = key[order], gsrc[order], dstl[order], vals[order]
    bounds = np.searchsorted(ks, np.arange(NCORE * TPC + 1))

    groups = {}
    nuniq = np.zeros((NCORE, TPC), np.int64)
    for c in range(NCORE):
        for t in range(TPC):
            k = c * TPC + t
            s, e = bounds[k], bounds[k + 1]
            uq, inv = np.unique(gs[s:e], return_inverse=True)
            groups[(c, t)] = (uq, inv, ds[s:e], vs[s:e])
            nuniq[c, t] = len(uq)
    nslab = [int(np.ceil(max(nuniq[:, t].max(), 1) / TILE)) for t in range(TPC)]

    idx_pc, m_pc = [], []
    for c in range(NCORE):
        toks, ms = [], []
        for t in range(TPC):
            uq, inv, dl, vv = groups[(c, t)]
            nt = nslab[t] * TILE
            tok = np.zeros(nt, np.int64)
            tok[:len(uq)] = uq
            Mf = np.zeros((nt, TILE), np.float32)
            np.add.at(Mf, (inv, dl), vv)
            toks.append(tok)
            ms.append(np.ascontiguousarray(
                Mf.reshape(nslab[t], TILE, TILE).transpose(1, 0, 2)
                .reshape(TILE, nslab[t] * TILE)).astype(np.float16))
        idx_pc.append(_pack_tok_idx(np.concatenate(toks)))
        m_pc.append(np.concatenate(ms, axis=1))
    return nslab, idx_pc, m_pc


def _host_plan(inputs):
    r0 = np.asarray(inputs['s0_rows']).astype(np.int64)
    c0 = np.asarray(inputs['s0_cols']).astype(np.int64)
    w0 = np.asarray(inputs['s0_vals'], np.float32)
    r1 = np.asarray(inputs['s1_rows']).astype(np.int64)
    c1 = np.asarray(inputs['s1_cols']).astype(np.int64)
    w1 = np.asarray(inputs['s1_vals'], np.float32)

    ns0, idx0, m0 = _build_support(r0, c0, w0)
    ns1, idx1, m1 = _build_support(r1, c1, w1)

    # folded weight blocks: order [W0-W2-W4, W1, 2W2, W3, 2W4] matches the
    # diffusion table order [x0, x1s0, x2s0(=S0^2 x0), x1s1, x2s1].
    # Feature rows permuted to [state(64), inputs(2)] so device partition
    # slices land on 32-aligned bases.
    FPERM = list(range(DIN, F)) + list(range(DIN))
    gwr = np.asarray(inputs['gate_w'], np.float32).reshape(F, NM, 2 * U)
    gwr = gwr[FPERM]
    gB = [gwr[:, 0] - gwr[:, 2] - gwr[:, 4], gwr[:, 1], 2.0 * gwr[:, 2],
          gwr[:, 3], 2.0 * gwr[:, 4]]
    cwr = np.asarray(inputs['cand_w'], np.float32).reshape(F, NM, U)
    cwr = cwr[FPERM]
    cB = [cwr[:, 0] - cwr[:, 2] - cwr[:, 4], cwr[:, 1], 2.0 * cwr[:, 2],
          cwr[:, 3], 2.0 * cwr[:, 4]]
    shared = {
        'gw': np.concatenate(gB, 1).astype(np.float16),       # [66, 640]
        'cwp': np.concatenate(cB, 1).astype(np.float16),      # [66, 320]
        'biasT': np.asarray(inputs['gate_b'],
                            np.float32).reshape(2 * U, 1),    # [128, 1]
        'ident': np.eye(128, dtype=np.float16),
    }
    return dict(ns0=ns0, ns1=ns1, idx0=idx0, idx1=idx1, m0=m0, m1=m1,
                shared=shared)


def _build_tables(inputs):
    inp = np.asarray(inputs['inputs'], np.float32).reshape(B, N, DIN)
    st = np.asarray(inputs['state'], np.float32).reshape(B, N, U)
    g = _grow(np.arange(N))

    # token feature order: [state(64), inputs(2), pad(6)]
    tok = np.zeros((FULL, B, FB), np.float32)
    tok[g, :, 0:U] = st.transpose(1, 0, 2)
    tok[g, :, U:F] = inp.transpose(1, 0, 2)
    x0tab8 = np.ascontiguousarray(tok.reshape(FULL, WG)).astype(F8NP)

    # per-core node-major state tokens (final GRU mix) + transposed x0
    stowns, x0Ts = [], []
    xT = np.zeros((F, FULL, B), np.float32)   # [f, grow, b]
    xT[0:U, g, :] = st.transpose(2, 1, 0)
    xT[U:F, g, :] = inp.transpose(2, 1, 0)
    stf = np.zeros((FULL, B, U), np.float16)
    stf[g] = st.transpose(1, 0, 2).astype(np.float16)
    for c in range(NCORE):
        rsel = np.concatenate(
            [np.arange((t // 5) * HALF + c * (5 * TILE) + (t % 5) * TILE,
                       (t // 5) * HALF + c * (5 * TILE) + (t % 5) * TILE
                       + TILE) for t in range(TPC)])
        stowns.append(np.ascontiguousarray(
            stf[rsel].reshape(RT, WC)))
        # x0T layout [F, (t, b, i)] so tile t slice is [F, B*TILE] contiguous
        xc = xT[:, rsel, :].reshape(F, TPC, TILE, B).transpose(0, 1, 3, 2)
        x0Ts.append(np.ascontiguousarray(
            xc.reshape(F, TPC * B * TILE)).astype(np.float16))
    return x0tab8, stowns, x0Ts


# ------------------------------------------------------------- device program
def _build_program(plan):
    import concourse.bacc as bacc
    import concourse.mybir as mybir
    from concourse.tile import TileContext
    from concourse.tile_rust import add_dep_helper

    f16 = mybir.dt.float16
    f32 = mybir.dt.float32
    f8 = mybir.dt.float8e3
    i16 = mybir.dt.int16
    ADD = mybir.AluOpType.add
    SUB = mybir.AluOpType.subtract
    MUL = mybir.AluOpType.mult
    BYP = mybir.AluOpType.bypass
    SIG = mybir.ActivationFunctionType.Sigmoid
    TANH = mybir.ActivationFunctionType.Tanh

    ns0, ns1 = plan['ns0'], plan['ns1']
    S0TOK = sum(ns0) * TILE
    S1TOK = sum(ns1) * TILE
    RG = [list(range(NCORE))]

    nc = bacc.Bacc('TRN2', target_bir_lowering=False, debug=False,
                   num_devices=NCORE)

    x0tab8_d = nc.dram_tensor('x0tab8', [FULL, WG], f8, kind='ExternalInput')
    stown_d = nc.dram_tensor('stown', [RT, WC], f16, kind='ExternalInput')
    x0T_d = nc.dram_tensor('x0T', [F, TPC * B * TILE], f16,
                           kind='ExternalInput')
    idx0_d = nc.dram_tensor('idx0', [128, S0TOK // 16], i16,
                            kind='ExternalInput')
    idx1_d = nc.dram_tensor('idx1', [128, S1TOK // 16], i16,
                            kind='ExternalInput')
    m0_d = nc.dram_tensor('m0', [128, S0TOK], f16, kind='ExternalInput')
    m1_d = nc.dram_tensor('m1', [128, S1TOK], f16, kind='ExternalInput')
    gw_d = nc.dram_tensor('gw', [F, NM * 2 * U], f16, kind='ExternalInput')
    cwp_d = nc.dram_tensor('cwp', [F, NM * U], f16, kind='ExternalInput')
    biasT_d = nc.dram_tensor('biasT', [2 * U, 1], f32, kind='ExternalInput')
    ident_d = nc.dram_tensor('ident', [128, 128], f16, kind='ExternalInput')
    out_d = nc.dram_tensor('out', [RT, B, U], f32, kind='ExternalOutput')

    def dram(name, shape, dt, shared=False):
        return nc.dram_tensor(name, shape, dt,
                              addr_space='Shared' if shared else 'Local')

    x1s0own = dram('x1s0own', [RT, WG], f16)
    x1s1own = dram('x1s1own', [RT, WG], f16)
    x2s0own = dram('x2s0own', [RT, WG], f16)
    x2s1own = dram('x2s1own', [RT, WG], f16)
    x1s0in8 = dram('x1s0in8', [RT, WG], f8)
    x1s1in8 = dram('x1s1in8', [RT, WG], f8)
    x1s0tab8 = dram('x1s0tab8', [FULL, WG], f8, shared=True)
    x1s1tab8 = dram('x1s1tab8', [FULL, WG], f8, shared=True)
    ypin8 = dram('ypin8', [RT, 2 * WC], f8)
    yptab8 = dram('yptab8', [FULL, 2 * WC], f8, shared=True)
    zin8 = dram('zin8', [RT, 2 * WC], f8)
    ztab8 = dram('ztab8', [FULL, 2 * WC], f8, shared=True)
    y0own = dram('y0own', [RT, WC], f16)
    y1own = dram('y1own', [RT, WC], f16)
    y3own = dram('y3own', [RT, WC], f16)
    ut_d = dram('ut', [RT, WC], f16)

    with TileContext(nc) as tc:
        with (
            tc.tile_pool(name='gp', bufs=3) as gp,
            tc.tile_pool(name='mp', bufs=2) as mp,
            tc.tile_pool(name='ev', bufs=2) as ev,
            tc.tile_pool(name='cst', bufs=1) as cst,
        ):
            idx0_sb = cst.tile([128, S0TOK // 16], i16, name='idx0')
            nc.sync.dma_start(idx0_sb[:], idx0_d[:])
            idx1_sb = cst.tile([128, S1TOK // 16], i16, name='idx1')
            nc.sync.dma_start(idx1_sb[:], idx1_d[:])
            gw = cst.tile([F, NM * 2 * U], f16, name='gw')
            nc.sync.dma_start(gw[:], gw_d[:])
            cwp = cst.tile([F, NM * U], f16, name='cwp')
            nc.sync.dma_start(cwp[:], cwp_d[:])
            biasT = cst.tile([2 * U, 1], f32, name='biasT')
            nc.sync.dma_start(biasT[:], biasT_d[:])
            ident = cst.tile([128, 128], f16, name='ident')
            nc.sync.dma_start(ident[:], ident_d[:])

            tab_ccs = {}     # table name -> list of AG insts (read deps)

            def ag(inb, outb, h, writes, key):
                hw = 5 * TILE
                cc = nc.gpsimd.collective_compute(
                    'AllGather', BYP, RG,
                    ins=[inb[h * hw:(h + 1) * hw, :].opt()],
                    outs=[outb[h * HALF:(h + 1) * HALF, :].opt()])
                for w in writes:
                    add_dep_helper(cc.ins, w.ins, reason='ag-input-write')
                tab_ccs.setdefault(key, []).append(cc)
                return cc

            def spmm_tile(pool, tag, tab_ap, idx_sb, m_d, nsl, off, elem,
                          estep=None, tab_key=None):
                """One tile's SpMM: gather fp8 rows, M-matmul into PSUM."""
                msb = mp.tile([128, nsl * TILE], f16, name='m', tag='m')
                nc.sync.dma_start(
                    msb[:], m_d[:, off * TILE:(off + nsl) * TILE])
                acc = pool.tile([128, elem], f32, name=tag, tag=tag)
                ch = (nsl + 1) // 2
                s0 = 0
                while s0 < nsl:
                    cn = min(ch, nsl - s0)
                    g = gp.tile([128, ch, elem], f8, name='g', tag='g')
                    gi = nc.gpsimd.dma_gather(
                        g[:, 0:cn, 0:elem], tab_ap,
                        idx_sb[:, (off + s0) * 8:(off + s0 + cn) * 8],
                        cn * TILE, cn * TILE, elem, elem_step=estep,
                        single_packet=False)
                    for cc in tab_ccs.get(tab_key, ()):
                        add_dep_helper(gi.ins, cc.ins, reason='gather-after-ag')
                    for s in range(cn):
                        sl = s0 + s
                        for w0 in range(0, elem, 512):
                            w1 = min(w0 + 512, elem)
                            nc.tensor.matmul(
                                acc[:, w0:w1],
                                msb[:, sl * TILE:(sl + 1) * TILE],
                                g[:, s, w0:w1],
                                start=(sl == 0), stop=(sl == nsl - 1),
                                skip_group_check=True)
                    s0 += cn
                return acc

            # ---- diffusion passes 1-4 (gate path) ----
            with tc.tile_pool(name='psA', bufs=1, space='PSUM') as psA:

                def xpass(tab_ap, idx_sb, m_d, nslab, own_dst, in8_dst,
                          tab_in_key, ag_out=None, ag_key=None):
                    off = 0
                    writes = []
                    for t in range(TPC):
                        acc = spmm_tile(psA, 'acc', tab_ap, idx_sb, m_d,
                                        nslab[t], off, WG, tab_key=tab_in_key)
                        off += nslab[t]
                        o16 = ev.tile([128, WG], f16, name='o16', tag='o16')
                        nc.vector.tensor_copy(o16[:], acc[:])
                        nc.sync.dma_start(
                            own_dst[t * TILE:(t + 1) * TILE, :], o16[:])
                        if in8_dst is not None:
                            o8 = ev.tile([128, WG], f8, name='o8', tag='o8')
                            nc.vector.tensor_copy(o8[:], acc[:])
                            writes.append(nc.sync.dma_start(
                                in8_dst[t * TILE:(t + 1) * TILE, :], o8[:]))
                        if ag_out is not None and t == 4:
                            ag(in8_dst, ag_out, 0, writes, ag_key)
                    if ag_out is not None:
                        ag(in8_dst, ag_out, 1, writes[5:], ag_key)

                xpass(x0tab8_d[:], idx0_sb, m0_d, ns0, x1s0own, x1s0in8,
                      None, x1s0tab8, 'x1s0')
                xpass(x0tab8_d[:], idx1_sb, m1_d, ns1, x1s1own, x1s1in8,
                      None, x1s1tab8, 'x1s1')
                xpass(x1s0tab8[:], idx0_sb, m0_d, ns0, x2s0own, None, 'x1s0')
                xpass(x1s1tab8[:], idx1_sb, m1_d, ns1, x2s1own, None, 'x1s1')

            # ---- gate + candidate projections (transposed domain) ----
            with (
                tc.tile_pool(name='xm', bufs=2) as xmp,
                tc.tile_pool(name='x0t', bufs=2) as x0tp,
                tc.tile_pool(name='xts', bufs=2) as xtsp,
                tc.tile_pool(name='prj', bufs=2) as prj,
                tc.tile_pool(name='ya', bufs=1) as yap,
                tc.tile_pool(name='pT', bufs=2, space='PSUM') as pT,
                tc.tile_pool(name='pg', bufs=2, space='PSUM') as pgp,
                tc.tile_pool(name='yc', bufs=2, space='PSUM') as ycp,
                tc.tile_pool(name='bp', bufs=2, space='PSUM') as bpp,
            ):
                yp_writes = []
                for t in range(TPC):
                    r0_, r1_ = t * TILE, (t + 1) * TILE
                    xms = []
                    for mi, src in enumerate(
                            (x1s0own, x2s0own, x1s1own, x2s1own)):
                        xt = xmp.tile([128, WG], f16, name=f'xm{mi}',
                                      tag=f'xm{mi}')
                        nc.sync.dma_start(xt[:], src[r0_:r1_, :])
                        xms.append(xt)
                    x0Tt = x0tp.tile([F, B * TILE], f16, name='x0T',
                                     tag='x0T')
                    nc.sync.dma_start(
                        x0Tt[:], x0T_d[:, t * B * TILE:(t + 1) * B * TILE])
                    y0a = yap.tile([128, B, U], f16, name='y0a', tag='y0a')
                    y1a = yap.tile([128, B, U], f16, name='y1a', tag='y1a')
                    y3a = yap.tile([128, B, U], f16, name='y3a', tag='y3a')
                    ua = yap.tile([128, B, U], f16, name='ua', tag='ua')
                    y2a = yap.tile([128, B, U], f8, name='y2a', tag='y2a')
                    y4a = yap.tile([128, B, U], f8, name='y4a', tag='y4a')

                    for q in range(B // 4):
                        b0 = 4 * q
                        xts0 = x0Tt[:, b0 * TILE:(b0 + 4) * TILE]
                        xts = [xts0]
                        for m in range(1, NM):
                            pt = pT.tile([F, 512], f32, name='pt', tag='pt')
                            for k in range(4):
                                nc.tensor.matmul(
                                    pt[:, k * 128:(k + 1) * 128],
                                    xms[m - 1][:, (b0 + k) * FB:
                                               (b0 + k) * FB + F],
                                    ident[:], start=True, stop=True)
                            xs = xtsp.tile([F, 512], f16, name=f'xts{m}',
                                           tag=f'xts{m}')
                            nc.vector.tensor_copy(xs[:], pt[:])
                            xts.append(xs)
                        pg = pgp.tile([128, 512], f32, name='pg', tag='pg')
                        for m in range(NM):
                            nc.tensor.matmul(
                                pg[:], gw[:, m * 128:(m + 1) * 128], xts[m],
                                start=(m == 0), stop=(m == NM - 1))
                        gt = prj.tile([128, 512], f16, name='gt', tag='gt')
                        nc.scalar.activation(gt[:], pg[:], SIG,
                                             bias=biasT[:])
                        xpT = prj.tile([F, 512], f16, name='xpT', tag='xpT')
                        nc.vector.tensor_tensor(
                            xpT[0:U, :], gt[0:U, :], xts0[0:U, :], op=MUL)
                        nc.vector.tensor_copy(xpT[U:F, :], xts0[U:F, :])
                        yc01p = ycp.tile([128, 512], f32, name='yc01p',
                                         tag='ycp')
                        nc.tensor.matmul(yc01p[:], cwp[:, 0:128], xpT[:],
                                         start=True, stop=True)
                        yc01 = prj.tile([128, 512], f16, name='yc01',
                                        tag='yc01')
                        nc.vector.tensor_copy(yc01[:], yc01p[:])
                        yc23p = ycp.tile([128, 512], f32, name='yc23p',
                                         tag='ycp')
                        nc.tensor.matmul(yc23p[:], cwp[:, 128:256], xpT[:],
                                         start=True, stop=True)
                        yc23 = prj.tile([128, 512], f16, name='yc23',
                                        tag='yc23')
                        nc.vector.tensor_copy(yc23[:], yc23p[:])
                        yc4p = ycp.tile([U, 512], f32, name='yc4p',
                                        tag='ycp')
                        nc.tensor.matmul(yc4p[:], cwp[:, 256:320], xpT[:],
                                         start=True, stop=True)
                        y4u = prj.tile([128, 512], f16, name='y4u',
                                       tag='y4u')
                        nc.vector.tensor_copy(y4u[0:U, :], yc4p[:])
                        nc.vector.tensor_copy(y4u[U:128, :], gt[U:128, :])

                        for (P, dA, dB) in ((yc01, y0a, y1a),
                                            (yc23, y2a, y3a),
                                            (y4u, y4a, ua)):
                            bp = bpp.tile([128, 512], f32, name='bp',
                                          tag='bp')
                            for k in range(4):
                                nc.tensor.matmul(
                                    bp[:, k * 128:(k + 1) * 128],
                                    P[:, k * 128:(k + 1) * 128],
                                    ident[:], start=True, stop=True)
                            bpv = bp[:].rearrange('p (k o) -> p k o', k=4)
                            nc.vector.tensor_copy(dA[:, b0:b0 + 4, :],
                                                  bpv[:, :, 0:U])
                            nc.vector.tensor_copy(dB[:, b0:b0 + 4, :],
                                                  bpv[:, :, U:2 * U])

                    for (src, dst) in ((y0a, y0own), (y1a, y1own),
                                       (y3a, y3own), (ua, ut_d)):
                        nc.sync.dma_start(
                            dst[r0_:r1_, :],
                            src[:].rearrange('p b u -> p (b u)'))
                    yp_writes.append(nc.sync.dma_start(
                        ypin8[r0_:r1_, 0:WC],
                        y2a[:].rearrange('p b u -> p (b u)')))
                    yp_writes.append(nc.sync.dma_start(
                        ypin8[r0_:r1_, WC:2 * WC],
                        y4a[:].rearrange('p b u -> p (b u)')))
                    if t == 4:
                        ag(ypin8, yptab8, 0, yp_writes, 'yp')
                ag(ypin8, yptab8, 1, yp_writes[10:], 'yp')

            # ---- cand diffusion: z0 = y1 + S0 y2' ; z1 = y3 + S1 y4' ----
            with (
                tc.tile_pool(name='psZ', bufs=1, space='PSUM') as psZ,
                tc.tile_pool(name='fz', bufs=2) as fz,
            ):
                z_writes = []
                off0 = off1 = 0
                for t in range(TPC):
                    r0_, r1_ = t * TILE, (t + 1) * TILE
                    for (tag, half, idx_sb, m_d, nsl, off, ysrc) in (
                            ('z0', 0, idx0_sb, m0_d, ns0[t], off0, y1own),
                            ('z1', 1, idx1_sb, m1_d, ns1[t], off1, y3own)):
                        acc = spmm_tile(
                            psZ, tag, yptab8[:, half * WC:(half + 1) * WC],
                            idx_sb, m_d, nsl, off, WC, estep=2 * WC,
                            tab_key='yp')
                        yl = fz.tile([128, WC], f16, name='yl',
                                     tag=f'yl{half}')
                        nc.sync.dma_start(yl[:], ysrc[r0_:r1_, :])
                        z8 = fz.tile([128, WC], f8, name='z8',
                                     tag=f'z8{half}')
                        nc.vector.tensor_tensor(z8[:], acc[:], yl[:], op=ADD)
                        z_writes.append(nc.sync.dma_start(
                            zin8[r0_:r1_, half * WC:(half + 1) * WC], z8[:]))
                    off0 += ns0[t]
                    off1 += ns1[t]
                    if t == 4:
                        ag(zin8, ztab8, 0, z_writes, 'z')
                ag(zin8, ztab8, 1, z_writes[10:], 'z')

            # ---- final: cand = tanh(y0 + S0 z0 + S1 z1), GRU mix ----
            with (
                tc.tile_pool(name='psF', bufs=1, space='PSUM') as psF,
                tc.tile_pool(name='fin', bufs=2) as fin,
            ):
                off0 = off1 = 0
                for t in range(TPC):
                    r0_, r1_ = t * TILE, (t + 1) * TILE
                    a8 = spmm_tile(psF, 'a8', ztab8[:, 0:WC], idx0_sb, m0_d,
                                   ns0[t], off0, WC, estep=2 * WC,
                                   tab_key='z')
                    a10 = spmm_tile(psF, 'a10', ztab8[:, WC:2 * WC], idx1_sb,
                                    m1_d, ns1[t], off1, WC, estep=2 * WC,
                                    tab_key='z')
                    off0 += ns0[t]
                    off1 += ns1[t]
                    y0l = fin.tile([128, WC], f16, name='y0l', tag='y0l')
                    nc.sync.dma_start(y0l[:], y0own[r0_:r1_, :])
                    utl = fin.tile([128, WC], f16, name='utl', tag='utl')
                    nc.sync.dma_start(utl[:], ut_d[r0_:r1_, :])
                    stl = fin.tile([128, WC], f16, name='stl', tag='stl')
                    nc.sync.dma_start(stl[:], stown_d[r0_:r1_, :])
                    cp = fin.tile([128, WC], f32, name='cp', tag='cp')
                    nc.vector.tensor_tensor(cp[:], a8[:], y0l[:], op=ADD)
                    nc.vector.tensor_tensor(cp[:], cp[:], a10[:], op=ADD)
                    cd = fin.tile([128, WC], f16, name='cd', tag='cd')
                    nc.scalar.activation(cd[:], cp[:], TANH)
                    # new = c + u*(state - c)
                    dd = fin.tile([128, WC], f16, name='dd', tag='dd')
                    nc.vector.tensor_tensor(dd[:], stl[:], cd[:], op=SUB)
                    nc.vector.tensor_tensor(dd[:], dd[:], utl[:], op=MUL)
                    oo = fin.tile([128, B, U], f32, name='oo', tag='oo')
                    nc.vector.tensor_tensor(
                        oo[:].rearrange('r b u -> r (b u)'), cd[:], dd[:],
                        op=ADD)
                    nc.sync.dma_start(out_d[r0_:r1_], oo[:])

    nc.compile()
    return nc


def _make_in_maps(plan, tables):
    x0tab8, stowns, x0Ts = tables
    sh = plan['shared']
    in_maps = []
    for c in range(NCORE):
        m = dict(sh)
        m['x0tab8'] = x0tab8
        m['stown'] = stowns[c]
        m['x0T'] = x0Ts[c]
        m['idx0'] = plan['idx0'][c]
        m['idx1'] = plan['idx1'][c]
        m['m0'] = plan['m0'][c]
        m['m1'] = plan['m1'][c]
        in_maps.append(m)
    return in_maps


# ------------------------------------------------------------------ kernel()
def kernel(**inputs):
    from concourse.bass_utils import run_bass_kernel_spmd

    key = 'prog'
    if key not in _CACHE:
        plan = _host_plan(inputs)
        nc = _build_program(plan)
        _CACHE[key] = (plan, nc)
    plan, nc = _CACHE[key]

    in_maps = _make_in_maps(plan, _build_tables(inputs))
    res = run_bass_kernel_spmd(nc, in_maps, core_ids=list(range(NCORE)))
    out = np.concatenate(
        [r['out'][:SHARD] for r in res.results], 0)          # [N, B, U]
    out = np.ascontiguousarray(out.transpose(1, 0, 2)).reshape(B, N * U)
    return (out, out)


# revision 13
# speedup vs baseline: 1.0502x; 1.0502x over previous
"""DCGRU cell Trainium2 kernel v4 (8 NeuronCores, node-sharded SpMM).

Core c owns dest nodes [1250c, 1250(c+1)) as 10 tiles of 128 (zero-padded).
Diffusion gathers read fp8(e3m4) token tables (token = [B=32, 72] for the
x-path = 2304 B/row, [B, 64] halves of a 4096 B/row table for the
projected cand path); M matrices stay fp16 (mixed-dtype matmuls, fp32
PSUM accumulate).  Token feature order is [state(64), inputs(2), pad] so
partition slices stay 32-aligned.

Transposed (feature-on-partition) copies of the diffused tables are
produced inside the DMA-bound diffusion passes via identity-matmuls
(fills PE gaps, keeps the HAM clock-gate warm); the projection phase
then runs stationary-weight N=512 matmuls in the transposed domain with
copies split between the vector and scalar engines.  Chebyshev 2x/-x0
terms fold into the gate/cand weight blocks.  Tables AllGather in 5
groups of 2 tiles to shrink the exposed tail before dependent passes.
"""
import sys

sys.path.insert(0, '/opt/trn_rl_repo')

import numpy as np
import ml_dtypes

F8NP = ml_dtypes.float8_e3m4

N = 10000
U = 64
DIN = 2
B = 32
NCORE = 8
SHARD = N // NCORE            # 1250
TILE = 128
TPC = 10                      # tiles per core (1250 -> 10x128 padded)
RT = TPC * TILE               # 1280 padded rows per shard
FULL = RT * NCORE             # 10240 table rows
F = DIN + U                   # 66
FB = 72                       # per-batch stride in x tokens (66 + 6 pad)
WG = B * FB                   # 2304 x-path token elems (fp8: 2304 B, %256)
WC = B * U                    # 2048 cand-path token elems
NM = 5
GT = 2                        # tiles per AllGather group
NG = TPC // GT                # 5 groups
GRP = NCORE * GT * TILE       # 2048 table rows per group
XT_W = B * TILE               # 4096 cols per tile of a transposed table

_CACHE = {}


def _grow(n):
    c = n // SHARD
    loc = n % SHARD
    t = loc // TILE
    return (t // GT) * GRP + c * (GT * TILE) + (t % GT) * TILE + loc % TILE


def _rowbase(c, t):
    return (t // GT) * GRP + c * (GT * TILE) + (t % GT) * TILE


def _pack_tok_idx(vec):
    flat = np.asarray(vec, np.int16)
    M = len(flat)
    assert M % 16 == 0
    i = np.arange(M)
    buf = np.zeros((128, M // 16), np.int16)
    for g in range(8):
        buf[(i % 16) + 16 * g, i // 16] = flat
    return buf


def _build_support(rows, cols, vals):
    """Per-core per-tile unique-source token lists + M matrices."""
    core = rows // SHARD
    loc = rows % SHARD
    tile = loc // TILE
    dstl = loc % TILE
    gsrc = _grow(cols)
    key = core * TPC + tile
    order = np.argsort(key, kind='stable')
    ks, gs, ds, vs = key[order], gsrc[order], dstl[order], vals[order]
    bounds = np.searchsorted(ks, np.arange(NCORE * TPC + 1))

    groups = {}
    nuniq = np.zeros((NCORE, TPC), np.int64)
    for c in range(NCORE):
        for t in range(TPC):
            k = c * TPC + t
            s, e = bounds[k], bounds[k + 1]
            uq, inv = np.unique(gs[s:e], return_inverse=True)
            groups[(c, t)] = (uq, inv, ds[s:e], vs[s:e])
            nuniq[c, t] = len(uq)
    nslab = [int(np.ceil(max(nuniq[:, t].max(), 1) / TILE)) for t in range(TPC)]

    idx_pc, m_pc = [], []
    for c in range(NCORE):
        toks, ms = [], []
        for t in range(TPC):
            uq, inv, dl, vv = groups[(c, t)]
            nt = nslab[t] * TILE
            tok = np.zeros(nt, np.int64)
            tok[:len(uq)] = uq
            Mf = np.zeros((nt, TILE), np.float32)
            np.add.at(Mf, (inv, dl), vv)
            toks.append(tok)
            ms.append(np.ascontiguousarray(
                Mf.reshape(nslab[t], TILE, TILE).transpose(1, 0, 2)
                .reshape(TILE, nslab[t] * TILE)).astype(np.float16))
        idx_pc.append(_pack_tok_idx(np.concatenate(toks)))
        m_pc.append(np.concatenate(ms, axis=1))
    return nslab, idx_pc, m_pc


def _host_plan(inputs):
    r0 = np.asarray(inputs['s0_rows']).astype(np.int64)
    c0 = np.asarray(inputs['s0_cols']).astype(np.int64)
    w0 = np.asarray(inputs['s0_vals'], np.float32)
    r1 = np.asarray(inputs['s1_rows']).astype(np.int64)
    c1 = np.asarray(inputs['s1_cols']).astype(np.int64)
    w1 = np.asarray(inputs['s1_vals'], np.float32)

    ns0, idx0, m0 = _build_support(r0, c0, w0)
    ns1, idx1, m1 = _build_support(r1, c1, w1)

    # folded weight blocks: order [W0-W2-W4, W1, 2W2, W3, 2W4] matches the
    # diffusion table order [x0, x1s0, x2s0(=S0^2 x0), x1s1, x2s1].
    # Feature rows permuted to [state(64), inputs(2)] so device partition
    # slices land on 32-aligned bases.
    FPERM = list(range(DIN, F)) + list(range(DIN))
    gwr = np.asarray(inputs['gate_w'], np.float32).reshape(F, NM, 2 * U)
    gwr = gwr[FPERM]
    gB = [gwr[:, 0] - gwr[:, 2] - gwr[:, 4], gwr[:, 1], 2.0 * gwr[:, 2],
          gwr[:, 3], 2.0 * gwr[:, 4]]
    cwr = np.asarray(inputs['cand_w'], np.float32).reshape(F, NM, U)
    cwr = cwr[FPERM]
    cB = [cwr[:, 0] - cwr[:, 2] - cwr[:, 4], cwr[:, 1], 2.0 * cwr[:, 2],
          cwr[:, 3], 2.0 * cwr[:, 4]]
    shared = {
        'gw': np.concatenate(gB, 1).astype(np.float16),       # [66, 640]
        'cwp': np.concatenate(cB, 1).astype(np.float16),      # [66, 320]
        'biasT': np.asarray(inputs['gate_b'],
                            np.float32).reshape(2 * U, 1),    # [128, 1]
        'ident': np.eye(128, dtype=np.float16),
    }
    return dict(ns0=ns0, ns1=ns1, idx0=idx0, idx1=idx1, m0=m0, m1=m1,
                shared=shared)


def _build_tables(inputs):
    inp = np.asarray(inputs['inputs'], np.float32).reshape(B, N, DIN)
    st = np.asarray(inputs['state'], np.float32).reshape(B, N, U)
    g = _grow(np.arange(N))

    # token feature order: [state(64), inputs(2), pad(6)]
    tok = np.zeros((FULL, B, FB), np.float32)
    tok[g, :, 0:U] = st.transpose(1, 0, 2)
    tok[g, :, U:F] = inp.transpose(1, 0, 2)
    x0tab8 = np.ascontiguousarray(tok.reshape(FULL, WG)).astype(F8NP)

    # per-core node-major state tokens (final GRU mix) + transposed x0
    stowns, x0Ts = [], []
    xT = np.zeros((F, FULL, B), np.float32)   # [f, grow, b]
    xT[0:U, g, :] = st.transpose(2, 1, 0)
    xT[U:F, g, :] = inp.transpose(2, 1, 0)
    stf = np.zeros((FULL, B, U), np.float16)
    stf[g] = st.transpose(1, 0, 2).astype(np.float16)
    for c in range(NCORE):
        rsel = np.concatenate(
            [np.arange(_rowbase(c, t), _rowbase(c, t) + TILE)
             for t in range(TPC)])
        stowns.append(np.ascontiguousarray(stf[rsel].reshape(RT, WC)))
        # x0T layout [F, (t, b, i)] so tile t slice is [F, B*TILE] contiguous
        xc = xT[:, rsel, :].reshape(F, TPC, TILE, B).transpose(0, 1, 3, 2)
        x0Ts.append(np.ascontiguousarray(
            xc.reshape(F, TPC * XT_W)).astype(np.float16))
    return x0tab8, stowns, x0Ts


# ------------------------------------------------------------- device program
def _build_program(plan):
    import concourse.bacc as bacc
    import concourse.mybir as mybir
    from concourse.tile import TileContext
    from concourse.tile_rust import add_dep_helper

    f16 = mybir.dt.float16
    f32 = mybir.dt.float32
    f8 = mybir.dt.float8e3
    i16 = mybir.dt.int16
    ADD = mybir.AluOpType.add
    SUB = mybir.AluOpType.subtract
    MUL = mybir.AluOpType.mult
    BYP = mybir.AluOpType.bypass
    SIG = mybir.ActivationFunctionType.Sigmoid
    TANH = mybir.ActivationFunctionType.Tanh
    COPY = mybir.ActivationFunctionType.Copy

    ns0, ns1 = plan['ns0'], plan['ns1']
    S0TOK = sum(ns0) * TILE
    S1TOK = sum(ns1) * TILE
    RG = [list(range(NCORE))]

    nc = bacc.Bacc('TRN2', target_bir_lowering=False, debug=False,
                   num_devices=NCORE)

    x0tab8_d = nc.dram_tensor('x0tab8', [FULL, WG], f8, kind='ExternalInput')
    stown_d = nc.dram_tensor('stown', [RT, WC], f16, kind='ExternalInput')
    x0T_d = nc.dram_tensor('x0T', [F, TPC * XT_W], f16, kind='ExternalInput')
    idx0_d = nc.dram_tensor('idx0', [128, S0TOK // 16], i16,
                            kind='ExternalInput')
    idx1_d = nc.dram_tensor('idx1', [128, S1TOK // 16], i16,
                            kind='ExternalInput')
    m0_d = nc.dram_tensor('m0', [128, S0TOK], f16, kind='ExternalInput')
    m1_d = nc.dram_tensor('m1', [128, S1TOK], f16, kind='ExternalInput')
    gw_d = nc.dram_tensor('gw', [F, NM * 2 * U], f16, kind='ExternalInput')
    cwp_d = nc.dram_tensor('cwp', [F, NM * U], f16, kind='ExternalInput')
    biasT_d = nc.dram_tensor('biasT', [2 * U, 1], f32, kind='ExternalInput')
    ident_d = nc.dram_tensor('ident', [128, 128], f16, kind='ExternalInput')
    out_d = nc.dram_tensor('out', [RT, B, U], f32, kind='ExternalOutput')

    def dram(name, shape, dt, shared=False):
        return nc.dram_tensor(name, shape, dt,
                              addr_space='Shared' if shared else 'Local')

    x1s0in8 = dram('x1s0in8', [RT, WG], f8)
    x1s1in8 = dram('x1s1in8', [RT, WG], f8)
    x1s0tab8 = dram('x1s0tab8', [FULL, WG], f8, shared=True)
    x1s1tab8 = dram('x1s1tab8', [FULL, WG], f8, shared=True)
    xT_ds = [dram(f'xdT{i}', [F, TPC * XT_W], f16) for i in range(4)]
    ypin8 = dram('ypin8', [RT, 2 * WC], f8)
    yptab8 = dram('yptab8', [FULL, 2 * WC], f8, shared=True)
    zin8 = dram('zin8', [RT, 2 * WC], f8)
    ztab8 = dram('ztab8', [FULL, 2 * WC], f8, shared=True)
    y0own = dram('y0own', [RT, WC], f16)
    y1own = dram('y1own', [RT, WC], f16)
    y3own = dram('y3own', [RT, WC], f16)
    ut_d = dram('ut', [RT, WC], f16)

    with TileContext(nc) as tc:
        with (
            tc.tile_pool(name='mp', bufs=2) as mp,
            tc.tile_pool(name='ev', bufs=2) as ev,
            tc.tile_pool(name='cst', bufs=1) as cst,
        ):
            gp = None
            idx0_sb = cst.tile([128, S0TOK // 16], i16, name='idx0')
            nc.sync.dma_start(idx0_sb[:], idx0_d[:])
            idx1_sb = cst.tile([128, S1TOK // 16], i16, name='idx1')
            nc.sync.dma_start(idx1_sb[:], idx1_d[:])
            gw = cst.tile([F, NM * 2 * U], f16, name='gw')
            nc.sync.dma_start(gw[:], gw_d[:])
            cwp = cst.tile([F, NM * U], f16, name='cwp')
            nc.sync.dma_start(cwp[:], cwp_d[:])
            biasT = cst.tile([2 * U, 1], f32, name='biasT')
            nc.sync.dma_start(biasT[:], biasT_d[:])
            ident = cst.tile([128, 128], f16, name='ident')
            nc.sync.dma_start(ident[:], ident_d[:])

            tab_ccs = {}     # table key -> list of AG insts (read deps)

            def ag(inb, outb, grp, writes, key):
                gw_ = GT * TILE
                cc = nc.gpsimd.collective_compute(
                    'AllGather', BYP, RG,
                    ins=[inb[grp * gw_:(grp + 1) * gw_, :].opt()],
                    outs=[outb[grp * GRP:(grp + 1) * GRP, :].opt()])
                for w in writes:
                    add_dep_helper(cc.ins, w.ins, reason='ag-input-write')
                tab_ccs.setdefault(key, []).append(cc)
                return cc

            def spmm_tile(pool, tag, tab_ap, idx_sb, m_d, nsl, off, elem,
                          estep=None, tab_key=None):
                """One tile's SpMM: gather fp8 rows, M-matmul into PSUM."""
                msb = mp.tile([128, nsl * TILE], f16, name='m', tag='m')
                nc.sync.dma_start(
                    msb[:], m_d[:, off * TILE:(off + nsl) * TILE])
                acc = pool.tile([128, elem], f32, name=tag, tag=tag)
                ch = (nsl + 1) // 2
                s0 = 0
                while s0 < nsl:
                    cn = min(ch, nsl - s0)
                    g = gp.tile([128, ch, elem], f8, name='g', tag='g')
                    gi = nc.gpsimd.dma_gather(
                        g[:, 0:cn, 0:elem], tab_ap,
                        idx_sb[:, (off + s0) * 8:(off + s0 + cn) * 8],
                        cn * TILE, cn * TILE, elem, elem_step=estep,
                        single_packet=False)
                    for cc in tab_ccs.get(tab_key, ()):
                        add_dep_helper(gi.ins, cc.ins, reason='gather-after-ag')
                    for s in range(cn):
                        sl = s0 + s
                        for w0 in range(0, elem, 512):
                            w1 = min(w0 + 512, elem)
                            nc.tensor.matmul(
                                acc[:, w0:w1],
                                msb[:, sl * TILE:(sl + 1) * TILE],
                                g[:, s, w0:w1],
                                start=(sl == 0), stop=(sl == nsl - 1),
                                skip_group_check=True)
                    s0 += cn
                return acc

            # ---- diffusion passes 1-4 (gate path) ----
            # each pass also emits the transposed fp16 table tile-by-tile
            # (identity-matmuls fill PE gaps while gathers stream).
            with (
                tc.tile_pool(name='psA', bufs=1, space='PSUM') as psA,
                tc.tile_pool(name='psT', bufs=2, space='PSUM') as psT,
                tc.tile_pool(name='xto', bufs=2) as xto,
                tc.tile_pool(name='gpA', bufs=3) as gp,
            ):

                def xpass(tab_ap, idx_sb, m_d, nslab, xT_dst, in8_dst,
                          tab_in_key, ag_out=None, ag_key=None):
                    off = 0
                    writes = []
                    for t in range(TPC):
                        acc = spmm_tile(psA, 'acc', tab_ap, idx_sb, m_d,
                                        nslab[t], off, WG, tab_key=tab_in_key)
                        off += nslab[t]
                        o16 = ev.tile([128, WG], f16, name='o16', tag='o16')
                        nc.vector.tensor_copy(o16[:], acc[:])
                        if in8_dst is not None:
                            o8 = ev.tile([128, WG], f8, name='o8', tag='o8')
                            nc.vector.tensor_copy(o8[:], acc[:])
                            writes.append(nc.sync.dma_start(
                                in8_dst[t * TILE:(t + 1) * TILE, :], o8[:]))
                        # transposed table tile: 32 identity-matmuls
                        xTt = xto.tile([F, XT_W], f16, name='xT', tag='xT')
                        for q in range(B // 4):
                            b0 = 4 * q
                            pt = psT.tile([F, 512], f32, name='pt', tag='pt')
                            for k in range(4):
                                nc.tensor.matmul(
                                    pt[:, k * 128:(k + 1) * 128],
                                    o16[:, (b0 + k) * FB:(b0 + k) * FB + F],
                                    ident[:], start=True, stop=True)
                            nc.scalar.activation(
                                xTt[:, b0 * TILE:(b0 + 4) * TILE], pt[:],
                                COPY)
                        nc.sync.dma_start(
                            xT_dst[:, t * XT_W:(t + 1) * XT_W], xTt[:])
                        if ag_out is not None and t % GT == GT - 1:
                            ag(in8_dst, ag_out, t // GT, writes, ag_key)

                xpass(x0tab8_d[:], idx0_sb, m0_d, ns0, xT_ds[0], x1s0in8,
                      None, x1s0tab8, 'x1s0')
                xpass(x0tab8_d[:], idx1_sb, m1_d, ns1, xT_ds[2], x1s1in8,
                      None, x1s1tab8, 'x1s1')
                xpass(x1s0tab8[:], idx0_sb, m0_d, ns0, xT_ds[1], None,
                      'x1s0')
                xpass(x1s1tab8[:], idx1_sb, m1_d, ns1, xT_ds[3], None,
                      'x1s1')

            # ---- gate + candidate projections (transposed domain) ----
            with (
                tc.tile_pool(name='xti', bufs=2) as xti,
                tc.tile_pool(name='prj', bufs=2) as prj,
                tc.tile_pool(name='ya', bufs=1) as yap,
                tc.tile_pool(name='pg', bufs=2, space='PSUM') as pgp,
                tc.tile_pool(name='yc', bufs=2, space='PSUM') as ycp,
                tc.tile_pool(name='bp', bufs=2, space='PSUM') as bpp,
            ):
                yp_writes = []
                for t in range(TPC):
                    r0_, r1_ = t * TILE, (t + 1) * TILE
                    xts_t = []
                    for mi, src in enumerate([x0T_d] + xT_ds):
                        xt = xti.tile([F, XT_W], f16, name=f'xt{mi}',
                                      tag=f'xt{mi}')
                        nc.sync.dma_start(
                            xt[:], src[:, t * XT_W:(t + 1) * XT_W])
                        xts_t.append(xt)
                    # diffusion order [x0, x1s0, x2s0, x1s1, x2s1]
                    xord = [xts_t[0], xts_t[1], xts_t[2], xts_t[3], xts_t[4]]
                    ya_all = yap.tile([128, 6, B, U], f16, name='ya',
                                      tag='ya')

                    for q in range(B // 4):
                        b0 = 4 * q
                        c0_, c1_ = b0 * TILE, (b0 + 4) * TILE
                        pg = pgp.tile([128, 512], f32, name='pg', tag='pg')
                        for m in range(NM):
                            nc.tensor.matmul(
                                pg[:], gw[:, m * 128:(m + 1) * 128],
                                xord[m][:, c0_:c1_],
                                start=(m == 0), stop=(m == NM - 1))
                        gt = prj.tile([128, 512], f16, name='gt', tag='gt')
                        nc.scalar.activation(gt[:], pg[:], SIG,
                                             bias=biasT[:])
                        x0s = xts_t[0][:, c0_:c1_]
                        xpT = prj.tile([F, 512], f16, name='xpT', tag='xpT')
                        nc.vector.tensor_tensor(
                            xpT[0:U, :], gt[0:U, :], x0s[0:U, :], op=MUL)
                        nc.vector.tensor_copy(xpT[U:F, :], x0s[U:F, :])
                        yc01p = ycp.tile([128, 512], f32, name='yc01p',
                                         tag='ycp')
                        nc.tensor.matmul(yc01p[:], cwp[:, 0:128], xpT[:],
                                         start=True, stop=True)
                        yc01 = prj.tile([128, 512], f16, name='yc01',
                                        tag='yc01')
                        nc.scalar.activation(yc01[:], yc01p[:], COPY)
                        yc23p = ycp.tile([128, 512], f32, name='yc23p',
                                         tag='ycp')
                        nc.tensor.matmul(yc23p[:], cwp[:, 128:256], xpT[:],
                                         start=True, stop=True)
                        yc23 = prj.tile([128, 512], f16, name='yc23',
                                        tag='yc23')
                        nc.scalar.activation(yc23[:], yc23p[:], COPY)
                        yc4p = ycp.tile([U, 512], f32, name='yc4p',
                                        tag='ycp')
                        nc.tensor.matmul(yc4p[:], cwp[:, 256:320], xpT[:],
                                         start=True, stop=True)
                        yc4 = prj.tile([U, 512], f16, name='yc4', tag='yc4')
                        nc.vector.tensor_copy(yc4[:], yc4p[:])

                        # back-transposes into node-major ya_all
                        # table order [y0, y1, y2, y3, y4, u]
                        for (P, ta) in ((yc01, 0), (yc23, 2)):
                            bp = bpp.tile([128, 512], f32, name='bp',
                                          tag='bp')
                            for k in range(4):
                                nc.tensor.matmul(
                                    bp[:, k * 128:(k + 1) * 128],
                                    P[:, k * 128:(k + 1) * 128],
                                    ident[:], start=True, stop=True)
                            nc.vector.tensor_copy(
                                ya_all[:, ta:ta + 2, b0:b0 + 4, :]
                                .rearrange('p t b u -> p b t u'),
                                bp[:].rearrange('p (b t u) -> p b t u',
                                                b=4, t=2))
                        # u: transpose full gt chunks, keep cols U:128
                        bpg = bpp.tile([128, 512], f32, name='bpg',
                                       tag='bp')
                        for k in range(4):
                            nc.tensor.matmul(
                                bpg[:, k * 128:(k + 1) * 128],
                                gt[:, k * 128:(k + 1) * 128],
                                ident[:], start=True, stop=True)
                        nc.vector.tensor_copy(
                            ya_all[:, 5, b0:b0 + 4, :],
                            bpg[:].rearrange('p (b ru) -> p b ru',
                                             b=4)[:, :, U:128])
                        bp4 = bpp.tile([128, 256], f32, name='bp4',
                                       tag='bp4')
                        for k in range(4):
                            nc.tensor.matmul(
                                bp4[:, k * U:(k + 1) * U],
                                yc4[:, k * 128:(k + 1) * 128],
                                ident[0:U, 0:U], start=True, stop=True)
                        nc.vector.tensor_copy(
                            ya_all[:, 4, b0:b0 + 4, :],
                            bp4[:].rearrange('p (b u) -> p b u', b=4))

                    for (ti, dst) in ((0, y0own), (1, y1own), (3, y3own),
                                      (5, ut_d)):
                        nc.sync.dma_start(
                            dst[r0_:r1_, :],
                            ya_all[:, ti].rearrange('p b u -> p (b u)'))
                    for (ti, col) in ((2, 0), (4, WC)):
                        y8 = prj.tile([128, WC], f8, name='y8', tag='y8')
                        nc.vector.tensor_copy(
                            y8[:], ya_all[:, ti].rearrange(
                                'p b u -> p (b u)'))
                        yp_writes.append(nc.sync.dma_start(
                            ypin8[r0_:r1_, col:col + WC], y8[:]))
                    if t % GT == GT - 1:
                        ag(ypin8, yptab8, t // GT, yp_writes, 'yp')

            # ---- cand diffusion: z0 = y1 + S0 y2' ; z1 = y3 + S1 y4' ----
            with (
                tc.tile_pool(name='psZ', bufs=1, space='PSUM') as psZ,
                tc.tile_pool(name='fz', bufs=2) as fz,
                tc.tile_pool(name='gpZ', bufs=3) as gp,
            ):
                z_writes = []
                off0 = off1 = 0
                for t in range(TPC):
                    r0_, r1_ = t * TILE, (t + 1) * TILE
                    for (tag, half, idx_sb, m_d, nsl, off, ysrc) in (
                            ('z0', 0, idx0_sb, m0_d, ns0[t], off0, y1own),
                            ('z1', 1, idx1_sb, m1_d, ns1[t], off1, y3own)):
                        acc = spmm_tile(
                            psZ, tag, yptab8[:, half * WC:(half + 1) * WC],
                            idx_sb, m_d, nsl, off, WC, estep=2 * WC,
                            tab_key='yp')
                        yl = fz.tile([128, WC], f16, name='yl',
                                     tag=f'yl{half}')
                        nc.sync.dma_start(yl[:], ysrc[r0_:r1_, :])
                        z8 = fz.tile([128, WC], f8, name='z8',
                                     tag=f'z8{half}')
                        nc.vector.tensor_tensor(z8[:], acc[:], yl[:], op=ADD)
                        z_writes.append(nc.sync.dma_start(
                            zin8[r0_:r1_, half * WC:(half + 1) * WC], z8[:]))
                    off0 += ns0[t]
                    off1 += ns1[t]
                    if t % GT == GT - 1:
                        ag(zin8, ztab8, t // GT, z_writes, 'z')

            # ---- final: cand = tanh(y0 + S0 z0 + S1 z1), GRU mix ----
            with (
                tc.tile_pool(name='psF', bufs=1, space='PSUM') as psF,
                tc.tile_pool(name='fin', bufs=2) as fin,
                tc.tile_pool(name='gpF', bufs=3) as gp,
            ):
                off0 = off1 = 0
                for t in range(TPC):
                    r0_, r1_ = t * TILE, (t + 1) * TILE
                    a8 = spmm_tile(psF, 'a8', ztab8[:, 0:WC], idx0_sb, m0_d,
                                   ns0[t], off0, WC, estep=2 * WC,
                                   tab_key='z')
                    a10 = spmm_tile(psF, 'a10', ztab8[:, WC:2 * WC], idx1_sb,
                                    m1_d, ns1[t], off1, WC, estep=2 * WC,
                                    tab_key='z')
                    off0 += ns0[t]
                    off1 += ns1[t]
                    y0l = fin.tile([128, WC], f16, name='y0l', tag='y0l')
                    nc.sync.dma_start(y0l[:], y0own[r0_:r1_, :])
                    utl = fin.tile([128, WC], f16, name='utl', tag='utl')
                    nc.sync.dma_start(utl[:], ut_d[r0_:r1_, :])
                    stl = fin.tile([128, WC], f16, name='stl', tag='stl')
                    nc.sync.dma_start(stl[:], stown_d[r0_:r1_, :])
                    cp = fin.tile([128, WC], f32, name='cp', tag='cp')
                    nc.vector.tensor_tensor(cp[:], a8[:], y0l[:], op=ADD)
                    nc.vector.tensor_tensor(cp[:], cp[:], a10[:], op=ADD)
                    cd = fin.tile([128, WC], f16, name='cd', tag='cd')
                    nc.scalar.activation(cd[:], cp[:], TANH)
                    # new = c + u*(state - c)
                    dd = fin.tile([128, WC], f16, name='dd', tag='dd')
                    nc.vector.tensor_tensor(dd[:], stl[:], cd[:], op=SUB)
                    nc.vector.tensor_tensor(dd[:], dd[:], utl[:], op=MUL)
                    oo = fin.tile([128, B, U], f32, name='oo', tag='oo')
                    nc.vector.tensor_tensor(
                        oo[:].rearrange('r b u -> r (b u)'), cd[:], dd[:],
                        op=ADD)
                    nc.sync.dma_start(out_d[r0_:r1_], oo[:])

    nc.compile()
    return nc


def _make_in_maps(plan, tables):
    x0tab8, stowns, x0Ts = tables
    sh = plan['shared']
    in_maps = []
    for c in range(NCORE):
        m = dict(sh)
        m['x0tab8'] = x0tab8
        m['stown'] = stowns[c]
        m['x0T'] = x0Ts[c]
        m['idx0'] = plan['idx0'][c]
        m['idx1'] = plan['idx1'][c]
        m['m0'] = plan['m0'][c]
        m['m1'] = plan['m1'][c]
        in_maps.append(m)
    return in_maps


# ------------------------------------------------------------------ kernel()
def kernel(**inputs):
    from concourse.bass_utils import run_bass_kernel_spmd

    key = 'prog'
    if key not in _CACHE:
        plan = _host_plan(inputs)
        nc = _build_program(plan)
        _CACHE[key] = (plan, nc)
    plan, nc = _CACHE[key]

    in_maps = _make_in_maps(plan, _build_tables(inputs))
    res = run_bass_kernel_spmd(nc, in_maps, core_ids=list(range(NCORE)))
    out = np.concatenate(
        [r['out'][:SHARD] for r in res.results], 0)          # [N, B, U]
    out = np.ascontiguousarray(out.transpose(1, 0, 2)).reshape(B, N * U)
    return (out, out)


# revision 18
# speedup vs baseline: 1.0810x; 1.0294x over previous
"""DCGRU cell Trainium2 kernel v4 (8 NeuronCores, node-sharded SpMM).

Core c owns dest nodes [1250c, 1250(c+1)) as 10 tiles of 128 (zero-padded).
Diffusion gathers read fp8(e3m4) token tables (token = [B=32, 72] for the
x-path = 2304 B/row, [B, 64] halves of a 4096 B/row table for the
projected cand path); M matrices stay fp16 (mixed-dtype matmuls, fp32
PSUM accumulate).  Token feature order is [state(64), inputs(2), pad] so
partition slices stay 32-aligned.

Transposed (feature-on-partition) copies of the diffused tables are
produced inside the DMA-bound diffusion passes via identity-matmuls
(fills PE gaps, keeps the HAM clock-gate warm); the projection phase
then runs stationary-weight N=512 matmuls in the transposed domain with
copies split between the vector and scalar engines.  Chebyshev 2x/-x0
terms fold into the gate/cand weight blocks.  Tables AllGather in 5
groups of 2 tiles to shrink the exposed tail before dependent passes.
"""
import sys

sys.path.insert(0, '/opt/trn_rl_repo')

import numpy as np
import ml_dtypes

F8NP = ml_dtypes.float8_e3m4

N = 10000
U = 64
DIN = 2
B = 32
NCORE = 8
SHARD = N // NCORE            # 1250
TILE = 128
TPC = 10                      # tiles per core (1250 -> 10x128 padded)
RT = TPC * TILE               # 1280 padded rows per shard
FULL = RT * NCORE             # 10240 table rows
F = DIN + U                   # 66
FB = 72                       # per-batch stride in x tokens (66 + 6 pad)
WG = B * FB                   # 2304 x-path token elems (fp8: 2304 B, %256)
WC = B * U                    # 2048 cand-path token elems
NM = 5
GT = 2                        # tiles per AllGather group
NG = TPC // GT                # 5 groups
GRP = NCORE * GT * TILE       # 2048 table rows per group
XT_W = B * TILE               # 4096 cols per tile of a transposed table

_CACHE = {}


def _grow(n):
    c = n // SHARD
    loc = n % SHARD
    t = loc // TILE
    return (t // GT) * GRP + c * (GT * TILE) + (t % GT) * TILE + loc % TILE


def _rowbase(c, t):
    return (t // GT) * GRP + c * (GT * TILE) + (t % GT) * TILE


def _pack_tok_idx(vec):
    flat = np.asarray(vec, np.int16)
    M = len(flat)
    assert M % 16 == 0
    i = np.arange(M)
    buf = np.zeros((128, M // 16), np.int16)
    for g in range(8):
        buf[(i % 16) + 16 * g, i // 16] = flat
    return buf


def _build_support(rows, cols, vals):
    """Per-core per-tile unique-source token lists + M matrices."""
    core = rows // SHARD
    loc = rows % SHARD
    tile = loc // TILE
    dstl = loc % TILE
    gsrc = _grow(cols)
    key = core * TPC + tile
    order = np.argsort(key, kind='stable')
    ks, gs, ds, vs = key[order], gsrc[order], dstl[order], vals[order]
    bounds = np.searchsorted(ks, np.arange(NCORE * TPC + 1))

    groups = {}
    nuniq = np.zeros((NCORE, TPC), np.int64)
    for c in range(NCORE):
        for t in range(TPC):
            k = c * TPC + t
            s, e = bounds[k], bounds[k + 1]
            uq, inv = np.unique(gs[s:e], return_inverse=True)
            groups[(c, t)] = (uq, inv, ds[s:e], vs[s:e])
            nuniq[c, t] = len(uq)
    nslab = [int(np.ceil(max(nuniq[:, t].max(), 1) / TILE)) for t in range(TPC)]

    idx_pc, m_pc = [], []
    for c in range(NCORE):
        toks, ms = [], []
        for t in range(TPC):
            uq, inv, dl, vv = groups[(c, t)]
            nt = nslab[t] * TILE
            tok = np.zeros(nt, np.int64)
            tok[:len(uq)] = uq
            Mf = np.zeros((nt, TILE), np.float32)
            np.add.at(Mf, (inv, dl), vv)
            toks.append(tok)
            ms.append(np.ascontiguousarray(
                Mf.reshape(nslab[t], TILE, TILE).transpose(1, 0, 2)
                .reshape(TILE, nslab[t] * TILE)).astype(np.float16))
        idx_pc.append(_pack_tok_idx(np.concatenate(toks)))
        m_pc.append(np.concatenate(ms, axis=1))
    return nslab, idx_pc, m_pc


def _host_plan(inputs):
    r0 = np.asarray(inputs['s0_rows']).astype(np.int64)
    c0 = np.asarray(inputs['s0_cols']).astype(np.int64)
    w0 = np.asarray(inputs['s0_vals'], np.float32)
    r1 = np.asarray(inputs['s1_rows']).astype(np.int64)
    c1 = np.asarray(inputs['s1_cols']).astype(np.int64)
    w1 = np.asarray(inputs['s1_vals'], np.float32)

    ns0, idx0, m0 = _build_support(r0, c0, w0)
    ns1, idx1, m1 = _build_support(r1, c1, w1)

    # folded weight blocks: order [W0-W2-W4, W1, 2W2, W3, 2W4] matches the
    # diffusion table order [x0, x1s0, x2s0(=S0^2 x0), x1s1, x2s1].
    # Feature rows permuted to [state(64), inputs(2)] so device partition
    # slices land on 32-aligned bases.
    FPERM = list(range(DIN, F)) + list(range(DIN))
    gwr = np.asarray(inputs['gate_w'], np.float32).reshape(F, NM, 2 * U)
    gwr = gwr[FPERM]
    gB = [gwr[:, 0] - gwr[:, 2] - gwr[:, 4], gwr[:, 1], 2.0 * gwr[:, 2],
          gwr[:, 3], 2.0 * gwr[:, 4]]
    cwr = np.asarray(inputs['cand_w'], np.float32).reshape(F, NM, U)
    cwr = cwr[FPERM]
    cB = [cwr[:, 0] - cwr[:, 2] - cwr[:, 4], cwr[:, 1], 2.0 * cwr[:, 2],
          cwr[:, 3], 2.0 * cwr[:, 4]]
    shared = {
        'gw': np.concatenate(gB, 1).astype(np.float16),       # [66, 640]
        'cwp': np.concatenate(cB, 1).astype(np.float16),      # [66, 320]
        'biasT': np.asarray(inputs['gate_b'],
                            np.float32).reshape(2 * U, 1),    # [128, 1]
        'ident': np.eye(128, dtype=np.float16),
    }
    return dict(ns0=ns0, ns1=ns1, idx0=idx0, idx1=idx1, m0=m0, m1=m1,
                shared=shared)


def _build_tables(inputs):
    inp = np.asarray(inputs['inputs'], np.float32).reshape(B, N, DIN)
    st = np.asarray(inputs['state'], np.float32).reshape(B, N, U)
    g = _grow(np.arange(N))

    # token feature order: [state(64), inputs(2), pad(6)]
    tok = np.zeros((FULL, B, FB), np.float32)
    tok[g, :, 0:U] = st.transpose(1, 0, 2)
    tok[g, :, U:F] = inp.transpose(1, 0, 2)
    x0tab8 = np.ascontiguousarray(tok.reshape(FULL, WG)).astype(F8NP)

    # per-core node-major state tokens (final GRU mix) + transposed x0
    stowns, x0Ts = [], []
    xT = np.zeros((F, FULL, B), np.float32)   # [f, grow, b]
    xT[0:U, g, :] = st.transpose(2, 1, 0)
    xT[U:F, g, :] = inp.transpose(2, 1, 0)
    stf = np.zeros((FULL, B, U), np.float16)
    stf[g] = st.transpose(1, 0, 2).astype(np.float16)
    for c in range(NCORE):
        rsel = np.concatenate(
            [np.arange(_rowbase(c, t), _rowbase(c, t) + TILE)
             for t in range(TPC)])
        stowns.append(np.ascontiguousarray(stf[rsel].reshape(RT, WC)))
        # x0T layout [F, (t, b, i)] so tile t slice is [F, B*TILE] contiguous
        xc = xT[:, rsel, :].reshape(F, TPC, TILE, B).transpose(0, 1, 3, 2)
        x0Ts.append(np.ascontiguousarray(
            xc.reshape(F, TPC * XT_W)).astype(np.float16))
    return x0tab8, stowns, x0Ts


# ------------------------------------------------------------- device program
def _build_program(plan):
    import concourse.bacc as bacc
    import concourse.mybir as mybir
    from concourse.tile import TileContext
    from concourse.tile_rust import add_dep_helper

    f16 = mybir.dt.float16
    f32 = mybir.dt.float32
    f8 = mybir.dt.float8e3
    i16 = mybir.dt.int16
    ADD = mybir.AluOpType.add
    SUB = mybir.AluOpType.subtract
    MUL = mybir.AluOpType.mult
    BYP = mybir.AluOpType.bypass
    SIG = mybir.ActivationFunctionType.Sigmoid
    TANH = mybir.ActivationFunctionType.Tanh
    COPY = mybir.ActivationFunctionType.Copy

    ns0, ns1 = plan['ns0'], plan['ns1']
    S0TOK = sum(ns0) * TILE
    S1TOK = sum(ns1) * TILE
    RG = [list(range(NCORE))]

    nc = bacc.Bacc('TRN2', target_bir_lowering=False, debug=False,
                   num_devices=NCORE)

    x0tab8_d = nc.dram_tensor('x0tab8', [FULL, WG], f8, kind='ExternalInput')
    stown_d = nc.dram_tensor('stown', [RT, WC], f16, kind='ExternalInput')
    x0T_d = nc.dram_tensor('x0T', [F, TPC * XT_W], f16, kind='ExternalInput')
    idx0_d = nc.dram_tensor('idx0', [128, S0TOK // 16], i16,
                            kind='ExternalInput')
    idx1_d = nc.dram_tensor('idx1', [128, S1TOK // 16], i16,
                            kind='ExternalInput')
    m0_d = nc.dram_tensor('m0', [128, S0TOK], f16, kind='ExternalInput')
    m1_d = nc.dram_tensor('m1', [128, S1TOK], f16, kind='ExternalInput')
    gw_d = nc.dram_tensor('gw', [F, NM * 2 * U], f16, kind='ExternalInput')
    cwp_d = nc.dram_tensor('cwp', [F, NM * U], f16, kind='ExternalInput')
    biasT_d = nc.dram_tensor('biasT', [2 * U, 1], f32, kind='ExternalInput')
    ident_d = nc.dram_tensor('ident', [128, 128], f16, kind='ExternalInput')
    out_d = nc.dram_tensor('out', [RT, B, U], f32, kind='ExternalOutput')

    def dram(name, shape, dt, shared=False):
        return nc.dram_tensor(name, shape, dt,
                              addr_space='Shared' if shared else 'Local')

    x1s0in8 = dram('x1s0in8', [RT, WG], f8)
    x1s1in8 = dram('x1s1in8', [RT, WG], f8)
    x1s0tab8 = dram('x1s0tab8', [FULL, WG], f8, shared=True)
    x1s1tab8 = dram('x1s1tab8', [FULL, WG], f8, shared=True)
    xT_ds = [dram(f'xdT{i}', [F, TPC * XT_W], f16) for i in range(4)]
    ypin8 = dram('ypin8', [RT, 2 * WC], f8)
    yptab8 = dram('yptab8', [FULL, 2 * WC], f8, shared=True)
    zin8 = dram('zin8', [RT, 2 * WC], f8)
    ztab8 = dram('ztab8', [FULL, 2 * WC], f8, shared=True)
    y0own = dram('y0own', [RT, WC], f16)
    y1own = dram('y1own', [RT, WC], f16)
    y3own = dram('y3own', [RT, WC], f16)
    ut_d = dram('ut', [RT, WC], f16)

    with TileContext(nc) as tc:
        with (
            tc.tile_pool(name='mp', bufs=2) as mp,
            tc.tile_pool(name='ev', bufs=2) as ev,
            tc.tile_pool(name='cst', bufs=1) as cst,
        ):
            gp = None
            idx0_sb = cst.tile([128, S0TOK // 16], i16, name='idx0')
            nc.sync.dma_start(idx0_sb[:], idx0_d[:])
            idx1_sb = cst.tile([128, S1TOK // 16], i16, name='idx1')
            nc.sync.dma_start(idx1_sb[:], idx1_d[:])
            gw = cst.tile([F, NM * 2 * U], f16, name='gw')
            nc.sync.dma_start(gw[:], gw_d[:])
            cwp = cst.tile([F, NM * U], f16, name='cwp')
            nc.sync.dma_start(cwp[:], cwp_d[:])
            biasT = cst.tile([2 * U, 1], f32, name='biasT')
            nc.sync.dma_start(biasT[:], biasT_d[:])
            ident = cst.tile([128, 128], f16, name='ident')
            nc.sync.dma_start(ident[:], ident_d[:])

            tab_ccs = {}     # table key -> list of AG insts (read deps)

            def ag(inb, outb, grp, writes, key):
                gw_ = GT * TILE
                cc = nc.gpsimd.collective_compute(
                    'AllGather', BYP, RG,
                    ins=[inb[grp * gw_:(grp + 1) * gw_, :].opt()],
                    outs=[outb[grp * GRP:(grp + 1) * GRP, :].opt()])
                for w in writes:
                    add_dep_helper(cc.ins, w.ins, reason='ag-input-write')
                tab_ccs.setdefault(key, []).append(cc)
                return cc

            def spmm_tile(pool, tag, tab_ap, idx_sb, m_d, nsl, off, elem,
                          estep=None, tab_key=None):
                """One tile's SpMM: gather fp8 rows, M-matmul into PSUM."""
                msb = mp.tile([128, nsl * TILE], f16, name='m', tag='m')
                nc.sync.dma_start(
                    msb[:], m_d[:, off * TILE:(off + nsl) * TILE])
                acc = pool.tile([128, elem], f32, name=tag, tag=tag)
                ch = (nsl + 1) // 2
                s0 = 0
                while s0 < nsl:
                    cn = min(ch, nsl - s0)
                    g = gp.tile([128, ch, elem], f8, name='g', tag='g')
                    gi = nc.gpsimd.dma_gather(
                        g[:, 0:cn, 0:elem], tab_ap,
                        idx_sb[:, (off + s0) * 8:(off + s0 + cn) * 8],
                        cn * TILE, cn * TILE, elem, elem_step=estep,
                        single_packet=False)
                    for cc in tab_ccs.get(tab_key, ()):
                        add_dep_helper(gi.ins, cc.ins, reason='gather-after-ag')
                    for s in range(cn):
                        sl = s0 + s
                        for w0 in range(0, elem, 512):
                            w1 = min(w0 + 512, elem)
                            nc.tensor.matmul(
                                acc[:, w0:w1],
                                msb[:, sl * TILE:(sl + 1) * TILE],
                                g[:, s, w0:w1],
                                start=(sl == 0), stop=(sl == nsl - 1),
                                skip_group_check=True)
                    s0 += cn
                return acc

            # ---- diffusion passes 1-4 (gate path) ----
            # each pass also emits the transposed fp16 table tile-by-tile
            # (identity-matmuls fill PE gaps while gathers stream).
            with (
                tc.tile_pool(name='psA', bufs=1, space='PSUM') as psA,
                tc.tile_pool(name='psT', bufs=2, space='PSUM') as psT,
                tc.tile_pool(name='xto', bufs=2) as xto,
                tc.tile_pool(name='gpA', bufs=4) as gp,
            ):

                def xpass(tab_ap, idx_sb, m_d, nslab, xT_dst, in8_dst,
                          tab_in_key, ag_out=None, ag_key=None):
                    off = 0
                    writes = []
                    for t in range(TPC):
                        acc = spmm_tile(psA, 'acc', tab_ap, idx_sb, m_d,
                                        nslab[t], off, WG, tab_key=tab_in_key)
                        off += nslab[t]
                        o16 = ev.tile([128, WG], f16, name='o16', tag='o16')
                        nc.vector.tensor_copy(o16[:], acc[:])
                        if in8_dst is not None:
                            o8 = ev.tile([128, WG], f8, name='o8', tag='o8')
                            nc.vector.tensor_copy(o8[:], acc[:])
                            writes.append(nc.sync.dma_start(
                                in8_dst[t * TILE:(t + 1) * TILE, :], o8[:]))
                        # transposed table tile: 32 identity-matmuls
                        xTt = xto.tile([F, XT_W], f16, name='xT', tag='xT')
                        for q in range(B // 4):
                            b0 = 4 * q
                            pt = psT.tile([F, 512], f32, name='pt', tag='pt')
                            for k in range(4):
                                nc.tensor.matmul(
                                    pt[:, k * 128:(k + 1) * 128],
                                    o16[:, (b0 + k) * FB:(b0 + k) * FB + F],
                                    ident[:], start=True, stop=True)
                            nc.scalar.activation(
                                xTt[:, b0 * TILE:(b0 + 4) * TILE], pt[:],
                                COPY)
                        nc.sync.dma_start(
                            xT_dst[:, t * XT_W:(t + 1) * XT_W], xTt[:])
                        if ag_out is not None and t % GT == GT - 1:
                            ag(in8_dst, ag_out, t // GT, writes, ag_key)

                xpass(x0tab8_d[:], idx0_sb, m0_d, ns0, xT_ds[0], x1s0in8,
                      None, x1s0tab8, 'x1s0')
                xpass(x0tab8_d[:], idx1_sb, m1_d, ns1, xT_ds[2], x1s1in8,
                      None, x1s1tab8, 'x1s1')
                xpass(x1s0tab8[:], idx0_sb, m0_d, ns0, xT_ds[1], None,
                      'x1s0')
                xpass(x1s1tab8[:], idx1_sb, m1_d, ns1, xT_ds[3], None,
                      'x1s1')

            # ---- gate + candidate projections (transposed domain) ----
            with (
                tc.tile_pool(name='xti', bufs=2) as xti,
                tc.tile_pool(name='prj', bufs=2) as prj,
                tc.tile_pool(name='ya', bufs=1) as yap,
                tc.tile_pool(name='pg', bufs=2, space='PSUM') as pgp,
                tc.tile_pool(name='yc', bufs=2, space='PSUM') as ycp,
                tc.tile_pool(name='bp', bufs=2, space='PSUM') as bpp,
            ):
                yp_writes = []
                for t in range(TPC):
                    r0_, r1_ = t * TILE, (t + 1) * TILE
                    xts_t = []
                    for mi, src in enumerate([x0T_d] + xT_ds):
                        xt = xti.tile([F, XT_W], f16, name=f'xt{mi}',
                                      tag=f'xt{mi}')
                        nc.sync.dma_start(
                            xt[:], src[:, t * XT_W:(t + 1) * XT_W])
                        xts_t.append(xt)
                    # diffusion order [x0, x1s0, x2s0, x1s1, x2s1]
                    xord = [xts_t[0], xts_t[1], xts_t[2], xts_t[3], xts_t[4]]
                    ya_all = yap.tile([128, 6, B, U], f16, name='ya',
                                      tag='ya')

                    for q in range(B // 4):
                        b0 = 4 * q
                        c0_, c1_ = b0 * TILE, (b0 + 4) * TILE
                        pg = pgp.tile([128, 512], f32, name='pg', tag='pg')
                        for m in range(NM):
                            nc.tensor.matmul(
                                pg[:], gw[:, m * 128:(m + 1) * 128],
                                xord[m][:, c0_:c1_],
                                start=(m == 0), stop=(m == NM - 1))
                        gt = prj.tile([128, 512], f16, name='gt', tag='gt')
                        nc.scalar.activation(gt[:], pg[:], SIG,
                                             bias=biasT[:])
                        x0s = xts_t[0][:, c0_:c1_]
                        xpT = prj.tile([F, 512], f16, name='xpT', tag='xpT')
                        nc.vector.tensor_tensor(
                            xpT[0:U, :], gt[0:U, :], x0s[0:U, :], op=MUL)
                        nc.vector.tensor_copy(xpT[U:F, :], x0s[U:F, :])
                        yc01p = ycp.tile([128, 512], f32, name='yc01p',
                                         tag='ycp')
                        nc.tensor.matmul(yc01p[:], cwp[:, 0:128], xpT[:],
                                         start=True, stop=True)
                        yc01 = prj.tile([128, 512], f16, name='yc01',
                                        tag='yc01')
                        nc.scalar.activation(yc01[:], yc01p[:], COPY)
                        yc23p = ycp.tile([128, 512], f32, name='yc23p',
                                         tag='ycp')
                        nc.tensor.matmul(yc23p[:], cwp[:, 128:256], xpT[:],
                                         start=True, stop=True)
                        yc23 = prj.tile([128, 512], f16, name='yc23',
                                        tag='yc23')
                        nc.scalar.activation(yc23[:], yc23p[:], COPY)
                        yc4p = ycp.tile([U, 512], f32, name='yc4p',
                                        tag='ycp')
                        nc.tensor.matmul(yc4p[:], cwp[:, 256:320], xpT[:],
                                         start=True, stop=True)
                        yc4 = prj.tile([U, 512], f16, name='yc4', tag='yc4')
                        nc.vector.tensor_copy(yc4[:], yc4p[:])

                        # back-transposes into node-major ya_all
                        # table order [y0, y1, y2, y3, y4, u]
                        for (P, ta) in ((yc01, 0), (yc23, 2)):
                            bp = bpp.tile([128, 512], f32, name='bp',
                                          tag='bp')
                            for k in range(4):
                                nc.tensor.matmul(
                                    bp[:, k * 128:(k + 1) * 128],
                                    P[:, k * 128:(k + 1) * 128],
                                    ident[:], start=True, stop=True)
                            nc.vector.tensor_copy(
                                ya_all[:, ta:ta + 2, b0:b0 + 4, :]
                                .rearrange('p t b u -> p b t u'),
                                bp[:].rearrange('p (b t u) -> p b t u',
                                                b=4, t=2))
                        # u: transpose full gt chunks, keep cols U:128
                        bpg = bpp.tile([128, 512], f32, name='bpg',
                                       tag='bp')
                        for k in range(4):
                            nc.tensor.matmul(
                                bpg[:, k * 128:(k + 1) * 128],
                                gt[:, k * 128:(k + 1) * 128],
                                ident[:], start=True, stop=True)
                        nc.vector.tensor_copy(
                            ya_all[:, 5, b0:b0 + 4, :],
                            bpg[:].rearrange('p (b ru) -> p b ru',
                                             b=4)[:, :, U:128])
                        bp4 = bpp.tile([128, 256], f32, name='bp4',
                                       tag='bp4')
                        for k in range(4):
                            nc.tensor.matmul(
                                bp4[:, k * U:(k + 1) * U],
                                yc4[:, k * 128:(k + 1) * 128],
                                ident[0:U, 0:U], start=True, stop=True)
                        nc.vector.tensor_copy(
                            ya_all[:, 4, b0:b0 + 4, :],
                            bp4[:].rearrange('p (b u) -> p b u', b=4))

                    for (ti, dst) in ((0, y0own), (1, y1own), (3, y3own),
                                      (5, ut_d)):
                        nc.sync.dma_start(
                            dst[r0_:r1_, :],
                            ya_all[:, ti].rearrange('p b u -> p (b u)'))
                    for (ti, col) in ((2, 0), (4, WC)):
                        y8 = prj.tile([128, WC], f8, name='y8', tag='y8')
                        nc.vector.tensor_copy(
                            y8[:], ya_all[:, ti].rearrange(
                                'p b u -> p (b u)'))
                        yp_writes.append(nc.sync.dma_start(
                            ypin8[r0_:r1_, col:col + WC], y8[:]))
                    if t % GT == GT - 1:
                        ag(ypin8, yptab8, t // GT, yp_writes, 'yp')

            # ---- cand diffusion: z0 = y1 + S0 y2' ; z1 = y3 + S1 y4' ----
            with (
                tc.tile_pool(name='psZ', bufs=1, space='PSUM') as psZ,
                tc.tile_pool(name='fz', bufs=2) as fz,
                tc.tile_pool(name='gpZ', bufs=4) as gp,
            ):
                z_writes = []
                off0 = off1 = 0
                for t in range(TPC):
                    r0_, r1_ = t * TILE, (t + 1) * TILE
                    for (tag, half, idx_sb, m_d, nsl, off, ysrc) in (
                            ('z0', 0, idx0_sb, m0_d, ns0[t], off0, y1own),
                            ('z1', 1, idx1_sb, m1_d, ns1[t], off1, y3own)):
                        acc = spmm_tile(
                            psZ, tag, yptab8[:, half * WC:(half + 1) * WC],
                            idx_sb, m_d, nsl, off, WC, estep=2 * WC,
                            tab_key='yp')
                        yl = fz.tile([128, WC], f16, name='yl',
                                     tag=f'yl{half}')
                        nc.sync.dma_start(yl[:], ysrc[r0_:r1_, :])
                        z8 = fz.tile([128, WC], f8, name='z8',
                                     tag=f'z8{half}')
                        nc.vector.tensor_tensor(z8[:], acc[:], yl[:], op=ADD)
                        z_writes.append(nc.sync.dma_start(
                            zin8[r0_:r1_, half * WC:(half + 1) * WC], z8[:]))
                    off0 += ns0[t]
                    off1 += ns1[t]
                    if t % GT == GT - 1:
                        ag(zin8, ztab8, t // GT, z_writes, 'z')

            # ---- final: cand = tanh(y0 + S0 z0 + S1 z1), GRU mix ----
            with (
                tc.tile_pool(name='psF', bufs=1, space='PSUM') as psF,
                tc.tile_pool(name='fin', bufs=2) as fin,
                tc.tile_pool(name='gpF', bufs=4) as gp,
            ):
                off0 = off1 = 0
                for t in range(TPC):
                    r0_, r1_ = t * TILE, (t + 1) * TILE
                    a8 = spmm_tile(psF, 'a8', ztab8[:, 0:WC], idx0_sb, m0_d,
                                   ns0[t], off0, WC, estep=2 * WC,
                                   tab_key='z')
                    a10 = spmm_tile(psF, 'a10', ztab8[:, WC:2 * WC], idx1_sb,
                                    m1_d, ns1[t], off1, WC, estep=2 * WC,
                                    tab_key='z')
                    off0 += ns0[t]
                    off1 += ns1[t]
                    y0l = fin.tile([128, WC], f16, name='y0l', tag='y0l')
                    nc.sync.dma_start(y0l[:], y0own[r0_:r1_, :])
                    utl = fin.tile([128, WC], f16, name='utl', tag='utl')
                    nc.sync.dma_start(utl[:], ut_d[r0_:r1_, :])
                    stl = fin.tile([128, WC], f16, name='stl', tag='stl')
                    nc.sync.dma_start(stl[:], stown_d[r0_:r1_, :])
                    cp = fin.tile([128, WC], f32, name='cp', tag='cp')
                    nc.vector.tensor_tensor(cp[:], a8[:], y0l[:], op=ADD)
                    nc.vector.tensor_tensor(cp[:], cp[:], a10[:], op=ADD)
                    cd = fin.tile([128, WC], f16, name='cd', tag='cd')
                    nc.scalar.activation(cd[:], cp[:], TANH)
                    # new = c + u*(state - c)
                    dd = fin.tile([128, WC], f16, name='dd', tag='dd')
                    nc.vector.tensor_tensor(dd[:], stl[:], cd[:], op=SUB)
                    nc.vector.tensor_tensor(dd[:], dd[:], utl[:], op=MUL)
                    oo = fin.tile([128, B, U], f32, name='oo', tag='oo')
                    nc.vector.tensor_tensor(
                        oo[:].rearrange('r b u -> r (b u)'), cd[:], dd[:],
                        op=ADD)
                    nc.sync.dma_start(out_d[r0_:r1_], oo[:])

    nc.compile()
    return nc


def _make_in_maps(plan, tables):
    x0tab8, stowns, x0Ts = tables
    sh = plan['shared']
    in_maps = []
    for c in range(NCORE):
        m = dict(sh)
        m['x0tab8'] = x0tab8
        m['stown'] = stowns[c]
        m['x0T'] = x0Ts[c]
        m['idx0'] = plan['idx0'][c]
        m['idx1'] = plan['idx1'][c]
        m['m0'] = plan['m0'][c]
        m['m1'] = plan['m1'][c]
        in_maps.append(m)
    return in_maps


# ------------------------------------------------------------------ kernel()
def kernel(**inputs):
    from concourse.bass_utils import run_bass_kernel_spmd

    key = 'prog'
    if key not in _CACHE:
        plan = _host_plan(inputs)
        nc = _build_program(plan)
        _CACHE[key] = (plan, nc)
    plan, nc = _CACHE[key]

    in_maps = _make_in_maps(plan, _build_tables(inputs))
    res = run_bass_kernel_spmd(nc, in_maps, core_ids=list(range(NCORE)))
    out = np.concatenate(
        [r['out'][:SHARD] for r in res.results], 0)          # [N, B, U]
    out = np.ascontiguousarray(out.transpose(1, 0, 2)).reshape(B, N * U)
    return (out, out)
